# revision 67
# baseline (speedup 1.0000x reference)
"""GaborNet Trainium2 kernel.

Math: per pixel p=(x1,x2) (the 2 input channels), per layer l, channel c:
  exp-arg  q_lc(p) = -0.5*||diag(gamma) R (p-mu)||^2   (quadratic in x1,x2)
  sin-arg  s_lc(p) = filt_w . p + filt_b               (linear)
  g_l = exp(q) * sin(s);  out_0 = g_0
  out_l = g_l * (W_{l-1} @ out_{l-1} + b_{l-1});  final = out_w @ out_4 + out_b

The quadratic form is expanded into 5 shared per-pixel features
[x1, x2, x1^2, x2^2, x1*x2] so that all per-channel math becomes matmuls
(PE), exp/sin run on the scalar engine (ACT), and only cheap elementwise
multiplies remain on the vector (DVE) / gpsimd (Pool) engines.

Layout: channels on partitions, pixels on the free dim. Tiles of T=2048
pixels are split into two groups of C=1024 stacked on partitions
(64ch x 2 groups = 128 partitions) via block-diagonal lhsT packing.

Features are built with a single full-width DVE multiply per tile:
the host stages two operand tensors u, v with rows chosen so that
u .* v = [x1, x2, x1^2, x2^2, x1*x2] per pixel group (v rows are 1.0
where a feature is just a passthrough of u).

Exp and Sin live in different ACT table sets, so tiles are processed
in super-tiles of ST=4 with ALTERNATING phase order (E0 S0 | S1 E1 |
E2 S2 | ...): consecutive super-tiles share a table boundary, halving
switches to 8+1 loads. Sin needs only the features, so whichever
function runs first has its outputs held; the second phase fires the
g-multiplies per layer. The per-tile MLP chain (bf16) is
software-pipelined through an ordered
action queue drained after every tile block — Exp blocks included, so
the DVE work spreads into phases where it would otherwise idle and the
serial chain never stalls the ACT-feeding PE stream. During the drain,
final bias-adds run on the then-idle ACT engine (Identity is in every
table set). Output staging DMAs ride the Pool/SWDGE queue.

Sharding: 8 cores x 65536 consecutive pixels (batch-major, then rows).
"""

import numpy as np

B, DIM, H, W = 2, 2, 512, 512
HID, OUT, NL = 64, 3, 4
NCORES = 8
NPIX = B * H * W // NCORES  # 65536 pixels per core
T = 2048                    # pixels per tile
C = T // 2                  # packed columns (2 pixel-groups on partitions)
NT = NPIX // T              # 32 tiles
ST = 4                      # tiles per super-tile (ACT table-switch batching)
MMCHUNK = 512               # fp32 moving-operand limit per matmul

_CACHE = {}

def _gabor_coeffs(filt_w, filt_b, mu, gamma, theta):
    """Host-side: per layer, coefficients of the exp-arg quadratic and the
    sin-arg linear on features [x1, x2, x1^2, x2^2, x1*x2], plus biases."""
    NL1 = theta.shape[0]
    Ge = np.zeros((NL1, 5, HID), np.float64)
    Gs = np.zeros((NL1, 5, HID), np.float64)
    be = np.zeros((NL1, HID), np.float64)
    bs = np.zeros((NL1, HID), np.float64)
    for l in range(NL1):
        ang = 2.0 * np.pi * theta[l].astype(np.float64)
        c, s = np.cos(ang), np.sin(ang)
        R = np.stack([np.stack([c, s], -1), np.stack([-s, c], -1)], -2)  # [64,2,2]
        A = gamma[l].astype(np.float64)[:, :, None] * R
        Q = np.einsum('coi,coj->cij', A, A)
        Qmu = np.einsum('cij,cj->ci', Q, mu[l].astype(np.float64))
        Ge[l, 0] = Qmu[:, 0]
        Ge[l, 1] = Qmu[:, 1]
        Ge[l, 2] = -0.5 * Q[:, 0, 0]
        Ge[l, 3] = -0.5 * Q[:, 1, 1]
        Ge[l, 4] = -Q[:, 0, 1]
        be[l] = -0.5 * np.einsum('ci,ci->c', mu[l].astype(np.float64), Qmu)
        Gs[l, 0] = filt_w[l, :, 0]
        Gs[l, 1] = filt_w[l, :, 1]
        bs[l] = filt_b[l]
    return Ge, Gs, be, bs


def _build_consts(filt_w, filt_b, mu, gamma, theta, lin_w, lin_b, out_w, out_b):
    import ml_dtypes
    Ge, Gs, be, bs = _gabor_coeffs(filt_w, filt_b, mu, gamma, theta)
    NL1 = NL + 1
    # gabor lhsT blocks: blocks 0..4 = exp layer l, 5..9 = sin layer l.
    # K rows 0-4: group A feats [x1, x2, x1^2, x2^2, x1x2]; rows 5-9: group B.
    gab = np.zeros((10, 10 * 128), np.float32)
    for l in range(NL1):
        for blk, G in ((l, Ge[l]), (5 + l, Gs[l])):
            gab[0:5, blk * 128:blk * 128 + 64] = G
            gab[5:10, blk * 128 + 64:blk * 128 + 128] = G
    # mlp lhsT blocks: diag(W^T, W^T)
    mlp = np.zeros((128, 4 * 128), np.float32)
    for l in range(NL):
        wT = lin_w[l].T.astype(np.float32)  # [in k, out m]
        mlp[0:64, l * 128:l * 128 + 64] = wT
        mlp[64:128, l * 128 + 64:l * 128 + 128] = wT
    # final lhsT: diag(out_w^T, out_w^T) -> [128, 6]
    fin = np.zeros((128, 6), np.float32)
    fin[0:64, 0:3] = out_w.T
    fin[64:128, 3:6] = out_w.T
    eb = np.concatenate([be, be], axis=1).T.astype(np.float32)    # [128, 5]
    sb = np.concatenate([bs, bs], axis=1).T.astype(np.float32)    # [128, 5]
    mb = np.concatenate([lin_b, lin_b], axis=1).T.astype(np.float32)  # [128, 4]
    ob = np.concatenate([out_b, out_b]).reshape(6, 1).astype(np.float32)
    mlp = mlp.astype(ml_dtypes.bfloat16)
    fin = fin.astype(ml_dtypes.bfloat16)
    return dict(gab=gab, mlp=mlp, fin=fin, eb=eb, sb=sb, mb=mb, ob=ob)


def _build_uv(xs):
    """Feature-multiply operands. xs: [2, NPIX] for one core.
    Per tile t, pixel groups A = [t*T, t*T+C), B = [t*T+C, (t+1)*T).
    u rows: [x1A, x2A, x1A, x2A, x1A, x1B, x2B, x1B, x2B, x1B]
    v rows: [1,   1,   x1A, x2A, x2A, 1,   1,   x1B, x2B, x2B]
    so u .* v = [x1, x2, x1^2, x2^2, x1*x2] per group, matching gab's
    K-row layout. Column c of tile t lives at u[:, t*C + c]."""
    xg = xs.reshape(2, NT, 2, C)  # [chan, tile, group, col]
    x1a, x2a = xg[0, :, 0], xg[1, :, 0]   # [NT, C]
    x1b, x2b = xg[0, :, 1], xg[1, :, 1]
    one = np.ones_like(x1a)
    u = np.stack([x1a, x2a, x1a, x2a, x1a, x1b, x2b, x1b, x2b, x1b], 0)
    v = np.stack([one, one, x1a, x2a, x2a, one, one, x1b, x2b, x2b], 0)
    return (np.ascontiguousarray(u.reshape(10, NT * C), np.float32),
            np.ascontiguousarray(v.reshape(10, NT * C), np.float32))


def _build_nc():
    import concourse.mybir as mybir
    import concourse.tile as tile
    from concourse import bacc

    f32 = mybir.dt.float32
    f32r = mybir.dt.float32r
    bf16 = mybir.dt.bfloat16
    AF = mybir.ActivationFunctionType
    ALU = mybir.AluOpType

    nc = bacc.Bacc("TRN2", target_bir_lowering=False, debug=False,
                   enable_asserts=False, num_devices=NCORES)

    u_d = nc.dram_tensor("u", [10, NT * C], f32r, kind="ExternalInput").ap()
    v_d = nc.dram_tensor("v", [10, NT * C], f32r, kind="ExternalInput").ap()
    gab_d = nc.dram_tensor("gab", [10, 10 * 128], f32r, kind="ExternalInput").ap()
    mlp_d = nc.dram_tensor("mlp", [128, 4 * 128], bf16, kind="ExternalInput").ap()
    fin_d = nc.dram_tensor("fin", [128, 6], bf16, kind="ExternalInput").ap()
    eb_d = nc.dram_tensor("eb", [128, 5], f32, kind="ExternalInput").ap()
    sb_d = nc.dram_tensor("sb", [128, 5], f32, kind="ExternalInput").ap()
    mb_d = nc.dram_tensor("mb", [128, 4], f32, kind="ExternalInput").ap()
    ob_d = nc.dram_tensor("ob", [6, 1], f32, kind="ExternalInput").ap()
    out_d = nc.dram_tensor("out", [3, NPIX], f32, kind="ExternalOutput").ap()

    def mm_pair(psum_ap, lhsT_ap, rhs_ap):
        for h in range(C // MMCHUNK):
            sl = slice(h * MMCHUNK, (h + 1) * MMCHUNK)
            nc.tensor.matmul(out=psum_ap[:, sl],
                             lhsT=lhsT_ap, rhs=rhs_ap[:, sl],
                             start=True, stop=True)

    with tile.TileContext(nc) as tc:
        with (
            tc.tile_pool(name="consts", bufs=1) as cpool,
            tc.tile_pool(name="feat", bufs=8) as fpool,
            tc.tile_pool(name="vv", bufs=2) as vpool,
            tc.tile_pool(name="pg", bufs=2, space="PSUM") as pg,
            tc.tile_pool(name="pm", bufs=4, space="PSUM") as pm,
            tc.tile_pool(name="epool", bufs=5 * ST + 2) as epool,
            tc.tile_pool(name="spool", bufs=5 * ST + 2) as spool,
            tc.tile_pool(name="gpool", bufs=18) as gpool,
            tc.tile_pool(name="opool", bufs=7) as opool,
            tc.tile_pool(name="obuf", bufs=4) as obpool,
        ):
            # gab + eb first: they gate the very first Exp activation.
            # Remaining consts ride behind the first super-tile's loads.
            eb = cpool.tile([128, 5], f32)
            nc.gpsimd.dma_start(out=eb, in_=eb_d)
            gab = cpool.tile([10, 10 * 128], f32r)
            nc.sync.dma_start(out=gab, in_=gab_d)
            mlp = cpool.tile([128, 4 * 128], bf16)
            fin = cpool.tile([128, 6], bf16)
            sb = cpool.tile([128, 5], f32)
            mb = cpool.tile([128, 4], f32)
            ob = cpool.tile([6, 1], f32)

            def load_late_consts():
                nc.sync.dma_start(out=sb, in_=sb_d)
                nc.sync.dma_start(out=mlp, in_=mlp_d)
                nc.sync.dma_start(out=mb, in_=mb_d)
                nc.sync.dma_start(out=fin, in_=fin_d)
                nc.sync.dma_start(out=ob, in_=ob_d)

            # chains[t] = [cur_tile, [g_0..g_4], next_layer] for the
            # in-flight MLP pipeline of tile t.
            chains = {}

            def mlp_step(t, l, drain=False):
                cur, gs, _ = chains[t]
                o = opool.tile([128, C], bf16, tag="o")
                for h in range(C // MMCHUNK):
                    sl = slice(h * MMCHUNK, (h + 1) * MMCHUNK)
                    pl = pm.tile([128, MMCHUNK], f32, tag="lin")
                    nc.tensor.matmul(out=pl,
                                     lhsT=mlp[:, (l - 1) * 128:l * 128],
                                     rhs=cur[:, sl], start=True, stop=True)
                    nc.vector.scalar_tensor_tensor(
                        out=o[:, sl], in0=pl, scalar=mb[:, l - 1:l],
                        in1=gs[l][:, sl], op0=ALU.add, op1=ALU.mult)
                chains[t][0] = o
                chains[t][2] = l + 1

            def advance(t, upto):
                if t in chains:
                    for l in range(chains[t][2], upto + 1):
                        mlp_step(t, l)

            def final_step(t, drain=False):
                cur, _, _ = chains.pop(t)
                # DRAM view [group, chan, col] so one DMA covers both
                # pixel groups; out_b is added while staging PSUM -> SBUF —
                # on DVE normally, on the otherwise-idle ACT engine during
                # the drain (Identity lives in every table set).
                ov = out_d[:, t * T:(t + 1) * T].rearrange(
                    "c (g p) -> g c p", p=C)
                osb = obpool.tile([6, C], f32, tag="osb")
                for h in range(C // MMCHUNK):
                    sl = slice(h * MMCHUNK, (h + 1) * MMCHUNK)
                    pf = pm.tile([128, MMCHUNK], f32, tag="lin")
                    nc.tensor.matmul(out=pf[0:6], lhsT=fin,
                                     rhs=cur[:, sl], start=True, stop=True)
                    if drain:
                        nc.scalar.activation(out=osb[:, sl], in_=pf[0:6],
                                             func=AF.Identity,
                                             bias=ob[:, 0:1])
                    else:
                        nc.vector.tensor_scalar_add(out=osb[:, sl],
                                                    in0=pf[0:6],
                                                    scalar1=ob[:, 0:1])
                (nc.sync if drain else nc.gpsimd).dma_start(out=ov, in_=osb)

            # Ordered queue of chain actions (t, layer|'F'). Actions are
            # emitted at "points" — one after each tile's Exp block and one
            # after each tile's Sin block — so the MLP work spreads evenly
            # over the whole schedule instead of bunching in Sin phases
            # (DVE would otherwise backlog there and trail after the last
            # activation).
            pending = []

            def pipeline_point(budget=4, drain=False):
                # emit up to `budget` actions, at most one per chain (the
                # next action of a chain must wait for the following point
                # so its dependency has a full point to complete)
                emitted = set()
                i = 0
                n = 0
                while i < len(pending) and n < budget:
                    t, a = pending[i]
                    if t in emitted:
                        i += 1
                        continue
                    pending.pop(i)
                    if t not in chains or (a != 'F' and chains[t][2] > a):
                        continue
                    emitted.add(t)
                    if a == 'F':
                        final_step(t, drain)
                    else:
                        mlp_step(t, a, drain)
                    n += 1

            feats = {}

            def load_feat(t, dma=None):
                if t >= NT:
                    return
                dma = dma or nc.sync
                lo = t * C
                u = fpool.tile([10, C], f32r, tag="u")
                dma.dma_start(out=u, in_=u_d[:, lo:lo + C])
                v = vpool.tile([10, C], f32r, tag="v")
                dma.dma_start(out=v, in_=v_d[:, lo:lo + C])
                nc.vector.tensor_mul(out=u, in0=u, in1=v)
                feats[t] = u

            for t in range(ST):
                load_feat(t)
            load_late_consts()

            # Sin activations depend only on the features, not on Exp —
            # only the g-multiplies need both. Alternating which function
            # runs first per super-tile (E0 S0 | S1 E1 | E2 S2 | S3 E3 ...)
            # halves the ACT table switches: 8 instead of 16. On the
            # super-tile's first phase, outputs are held; on its second,
            # the g-multiplies fire per layer as both operands exist.
            es, ss = {}, {}

            def gab_phase(st, kind):
                second = (st % 2 == 0) == (kind == "S")
                for tt in range(ST):
                    t = st * ST + tt
                    rhs = feats[t]
                    g_tiles = [None] * 5
                    for l in range(5):
                        ps = pg.tile([128, C], f32, tag="parg")
                        if kind == "E":
                            mm_pair(ps, gab[:, l * 128:(l + 1) * 128], rhs)
                            o = epool.tile([128, C], bf16, tag="e")
                            nc.scalar.activation(out=o, in_=ps, func=AF.Exp,
                                                 bias=eb[:, l:l + 1])
                            es[(t, l)] = o
                        else:
                            mm_pair(ps, gab[:, (5 + l) * 128:(6 + l) * 128],
                                    rhs)
                            o = spool.tile([128, C], bf16, tag="s")
                            nc.scalar.activation(out=o, in_=ps, func=AF.Sin,
                                                 bias=sb[:, l:l + 1])
                            ss[(t, l)] = o
                        if second:
                            g = gpool.tile([128, C], bf16, tag="g")
                            eng = nc.gpsimd if l >= 3 else nc.vector
                            eng.tensor_mul(out=g, in0=es.pop((t, l)),
                                           in1=ss.pop((t, l)))
                            if l == 0:
                                chains[t] = [g, g_tiles, 1]
                            g_tiles[l] = g
                    if second:
                        feats.pop(t)
                        # prefetch next super-tile's features before
                        # queueing MLP work so DVE never delays the
                        # ACT-feeding path
                        load_feat(t + ST)
                        pipeline_point()
                        pending.extend([(t, 1), (t, 2), (t, 3), (t, 4),
                                        (t, 'F')])
                    else:
                        pipeline_point()

            for st in range(NT // ST):
                if st % 2 == 0:
                    gab_phase(st, "E")
                    gab_phase(st, "S")
                else:
                    gab_phase(st, "S")
                    gab_phase(st, "E")

            # drain the MLP pipeline
            for _ in range(5 * ST):
                if not pending:
                    break
                pipeline_point(budget=4, drain=True)
            assert not pending and not chains
    nc.compile()
    return nc


def _get_nc():
    if "nc" not in _CACHE:
        _CACHE["nc"] = _build_nc()
    return _CACHE["nc"]


def _in_maps(x, consts):
    maps = []
    rows = H // (NCORES // B)  # 128 rows per core
    for k in range(NCORES):
        b, r = k // (NCORES // B), (k % (NCORES // B)) * rows
        xs = np.ascontiguousarray(
            x[b, :, r:r + rows, :].reshape(2, NPIX), np.float32)
        u, v = _build_uv(xs)
        m = {"u": u, "v": v}
        m.update(consts)
        maps.append(m)
    return maps


def _assemble(results):
    rows = H // (NCORES // B)
    out = np.empty((B, OUT, H, W), np.float32)
    for k in range(NCORES):
        b, r = k // (NCORES // B), (k % (NCORES // B)) * rows
        out[b, :, r:r + rows, :] = results[k]["out"].reshape(OUT, rows, W)
    return out


def run(x, filt_w, filt_b, mu, gamma, theta, lin_w, lin_b, out_w, out_b,
        trace=False):
    from concourse.bass_utils import run_bass_kernel_spmd
    nc = _get_nc()
    consts = _build_consts(np.asarray(filt_w), np.asarray(filt_b),
                           np.asarray(mu), np.asarray(gamma),
                           np.asarray(theta), np.asarray(lin_w),
                           np.asarray(lin_b), np.asarray(out_w),
                           np.asarray(out_b))
    maps = _in_maps(np.asarray(x), consts)
    res = run_bass_kernel_spmd(nc, maps, core_ids=list(range(NCORES)),
                               trace=trace)
    return _assemble(res.results), res


def kernel(**inputs):
    out, _ = run(**inputs)
    return out
